# revision 34
# baseline (speedup 1.0000x reference)
"""LITv1 transformer block on 8 TRN2 NeuronCores, data-parallel over batch.

v2: fp8 DoubleRow matmuls with error-feedback dual-fp8 operands.

Per-core layout (8 batches x 256 tokens):
- x/r1 residual stream resident in SBUF f32 (one 8MB buffer, updated in place)
- LN stats batched per phase (one Act Sqrt per phase -> few act-table loads)
- QKV: x-dual fp8 DoubleRow (xn hi+lo pairs in the moving slots, single-fp8
  weights broadcast stride-0 into the stationary slots) -> 0.25 cyc/k-tile
- attention internals bf16: S^T = K^T.T @ Q^T with the relative-position bias
  accumulated into PSUM via a dual-fp8 DoubleRow 8*I matmul; exp on Act ->
  bf16 P; token-major AV with a ones-column for the softmax denominator
- proj: o-dual fp8 DoubleRow stationary, single-fp8 weights broadcast
- MLP: fc1 both-dual via 3 single-DR passes (w_hi@x_hi + w_lo@x_hi +
  w_hi@x_lo); gelu -> fp8 h; fc2 w-dual DoubleRow (W2 hi+lo streamed)
- all transposes in bf16 via PE identity matmuls; dual-fp8 splits happen at
  the PSUM evacuation (hi = copy, lo = psum - hi via scalar_tensor_tensor)
- software-pipelined phase A (stage3 of batch b-1 between stage1/stage2 of
  batch b) so PE never waits on Act exps or DVE evacuations
"""
import sys

import numpy as np

sys.path.insert(0, "/opt/trn_rl_repo")

import ml_dtypes  # noqa: E402

import concourse.bass as bass  # noqa: E402
import concourse.mybir as mybir  # noqa: E402
import concourse.tile as tile  # noqa: E402
from concourse import bacc  # noqa: E402
from concourse.bass_utils import run_bass_kernel_spmd  # noqa: E402
from concourse.masks import make_identity  # noqa: E402

F32 = mybir.dt.float32
F8 = mybir.dt.float8e4
BF16 = mybir.dt.bfloat16
AF = mybir.ActivationFunctionType
ALU = mybir.AluOpType
PM = mybir.MatmulPerfMode

NP8 = ml_dtypes.float8_e4m3
NPB = ml_dtypes.bfloat16

B, N, C = 64, 256, 1024
H, DH = 16, 64
DFF = 4 * C
NCORES = 8
BLOC = B // NCORES          # 8 batches per core
TOK = BLOC * N              # 2048 tokens per core
NT = TOK // 128             # 16 token tiles of 128
KC = C // 128               # 8 contraction chunks of 128
KF = DFF // 128             # 32 dff chunks


def _pair(ap):
    """Broadcast a [128, X] AP to [128, 2, X] with stride-0 pair dim."""
    return ap.unsqueeze(1).broadcast_to([ap.shape[0], 2, ap.shape[-1]])


def build():
    nc = bacc.Bacc("TRN2")
    x_d = nc.dram_tensor("x", [TOK, C], F32, kind="ExternalInput")
    wqkv_d = nc.dram_tensor("wqkv", [128, KC, 3 * C], F8, kind="ExternalInput")
    wp_d = nc.dram_tensor("wp", [128, KC, C], F8, kind="ExternalInput")
    bt_d = nc.dram_tensor("bt", [128, 2, 2, H, N], F8, kind="ExternalInput")
    w1hi_d = nc.dram_tensor("w1hi", [128, KC, DFF], F8, kind="ExternalInput")
    w1lo_d = nc.dram_tensor("w1lo", [128, KC, DFF], F8, kind="ExternalInput")
    w2_d = nc.dram_tensor("w2", [128, KF, 2, C], F8, kind="ExternalInput")
    y_d = nc.dram_tensor("y", [TOK, C], F32, kind="ExternalOutput")

    with tile.TileContext(nc) as tc:
        with (
            tc.tile_pool(name="consts", bufs=1) as consts,
            tc.tile_pool(name="resid", bufs=1) as resid,
        ):
            ident_f = consts.tile([128, 128], F32)
            make_identity(nc, ident_f)
            identb = consts.tile([128, 128], BF16)
            nc.vector.tensor_copy(identb, ident_f)
            eight8 = consts.tile([128, 128], F8)
            nc.vector.tensor_scalar(eight8, ident_f, 8.0, None, ALU.mult)
            eps_sb = consts.tile([128, 1], F32)
            nc.vector.memset(eps_sb, 1e-5)

            xr = resid.tile([128, NT, C], F32)      # x then r1 then y, in place
            mvs = resid.tile([128, NT, 2], F32)     # LN1 mean/var
            rstds = resid.tile([128, NT], F32)
            mvs2 = resid.tile([128, NT, 2], F32)    # LN2 mean/var
            rstds2 = resid.tile([128, NT], F32)

            # ---------------- prologue: load x, LN1 stats --------------------
            pst_ctx = tc.tile_pool(name="pst", bufs=2)
            pst = pst_ctx.__enter__()
            sq = pst.tile([128, NT], F32, tag="sq")

            def ln1_stats(lo, hi_t):
                for t in range(lo, hi_t):
                    st = pst.tile([128, 2, 6], F32, tag="st", name="st")
                    xv = xr[:, t, :].rearrange("p (s f) -> p s f", s=2)
                    for s in range(2):
                        nc.vector.bn_stats(st[:, s, :], xv[:, s, :])
                    nc.vector.bn_aggr(mvs[:, t, :], st)
                nc.scalar.activation(
                    sq[:, lo:hi_t], mvs[:, lo:hi_t, 1], AF.Sqrt,
                    bias=eps_sb, scale=1.0,
                )
                nc.vector.reciprocal(rstds[:, lo:hi_t], sq[:, lo:hi_t])

            # first 2 x tiles unblock batch 0 quickly
            for t in range(2):
                nc.sync.dma_start(xr[:, t, :], x_d[t * 128:(t + 1) * 128, :])
            ln1_stats(0, 2)

            # ---------------- phase A: attention + proj ----------------------
            with (
                tc.tile_pool(name="paw", bufs=1) as paw,
                tc.tile_pool(name="pa1", bufs=2) as pa1,
                tc.tile_pool(name="pa2", bufs=2) as pa2,
                tc.tile_pool(name="pae", bufs=16) as pae,
                tc.tile_pool(name="psT", bufs=2, space="PSUM") as psT,
                tc.tile_pool(name="psQK", bufs=1, space="PSUM") as psQK,
                tc.tile_pool(name="psMM", bufs=1, space="PSUM") as psMM,
                tc.tile_pool(name="psS", bufs=2, space="PSUM") as psS,
                tc.tile_pool(name="psAV", bufs=2, space="PSUM") as psAV,
            ):
                # DMA order: wqkv before remaining x tiles so batch 0's QKV
                # isn't blocked; bt before wp (bias needed earlier than proj)
                wqkv_sb = paw.tile([128, KC, 3 * C], F8)
                nc.sync.dma_start(wqkv_sb, wqkv_d[:])
                bt_sb = paw.tile([128, 2, 2, H, N], F8)
                nc.sync.dma_start(bt_sb, bt_d[:])
                wp_sb = paw.tile([128, KC, C], F8)
                nc.sync.dma_start(wp_sb, wp_d[:])
                for t in range(2, NT):
                    nc.sync.dma_start(xr[:, t, :], x_d[t * 128:(t + 1) * 128, :])

                def tp4(pool_tag, srcs, ident):
                    """4 transposes into one PSUM bank as ONE accumulation
                    group (hardware zeroes PSUM lazily per 2KB region; separate
                    groups in one region would wipe earlier sub-tiles)."""
                    tp = psT.tile([128, 4, 128], BF16, tag=pool_tag)
                    for j, src in enumerate(srcs):
                        nc.tensor.matmul(
                            tp[:, j, :], src, ident, is_transpose=True,
                            start=(j == 0), stop=(j == 3),
                        )
                    return tp

                def stage1(b):
                    """LN1 norm/transpose/dual-evac + QKV(q,k) matmuls."""
                    xn16 = pa1.tile([128, 2, C], BF16, tag="xn16", name="xn16")
                    xnT = pa1.tile([128, KC, 2, N], F8, tag="xnT", name="xnT")
                    for i in range(2):
                        t = 2 * b + i
                        nc.vector.tensor_scalar(
                            xn16[:, i, :], xr[:, t, :],
                            mvs[:, t, 0:1], rstds[:, t:t + 1],
                            ALU.subtract, ALU.mult,
                        )
                        for cg in range(2):
                            tp = tp4("tp", [
                                xn16[:, i, (cg * 4 + j) * 128:(cg * 4 + j + 1) * 128]
                                for j in range(4)
                            ], identb)
                            hi = xnT[:, cg * 4:(cg + 1) * 4, 0, i * 128:(i + 1) * 128]
                            nc.scalar.copy(hi, tp)
                            nc.vector.scalar_tensor_tensor(
                                xnT[:, cg * 4:(cg + 1) * 4, 1, i * 128:(i + 1) * 128],
                                tp, 1.0, hi, ALU.mult, ALU.subtract,
                            )

                    qkT = pa2.tile([128, 2 * KC, N], BF16, tag="qkT", name="qkT")
                    for fp in range(KC):       # pairs of 128-feat chunks (Q,K)
                        qp = psQK.tile([128, 2, N], F32, tag="qk", name="qp")
                        for s in range(2):
                            fo = 2 * fp + s
                            for k in range(KC):
                                nc.tensor.matmul(
                                    qp[:, s, :],
                                    _pair(wqkv_sb[:, k, fo * 128:(fo + 1) * 128]),
                                    xnT[:, k, :, :],
                                    start=(s == 0 and k == 0),
                                    stop=(s == 1 and k == KC - 1),
                                    perf_mode=PM.DoubleRow,
                                )
                        nc.scalar.activation(
                            qkT[:, 2 * fp:2 * fp + 2, :], qp, AF.Copy,
                            scale=1.0 / 64,
                        )
                    return xnT, qkT

                def stage2(b, xnT, qkT):
                    """S^T + dual-fp8 bias + exp -> bf16 P, with the V matmuls
                    interleaved so PE has work while Act drains the exps."""
                    es = []
                    v_sb = pa2.tile([128, 2, H, DH + 1], BF16, tag="v", name="v_sb")
                    nc.vector.memset(v_sb[:, :, :, DH:DH + 1], 1.0)

                    def s_head(h):
                        sp = psS.tile([128, 2, N], F32, tag="sp", name="sp")
                        r0 = (h % 2) * 64
                        for nk in range(2):
                            nc.tensor.matmul(
                                sp[:, nk, :], _pair(eight8),
                                bt_sb[:, :, nk, h, :],
                                start=(nk == 0), stop=False,
                                perf_mode=PM.DoubleRow,
                            )
                            nc.tensor.matmul(
                                sp[:, nk, :],
                                qkT[r0:r0 + 64, KC + h // 2, nk * 128:(nk + 1) * 128],
                                qkT[r0:r0 + 64, h // 2, :],
                                start=False, stop=(nk == 1),
                            )
                        e = pae.tile([128, 2, N], BF16, tag="e", name="e")
                        nc.scalar.activation(e, sp, AF.Exp, bias=0.0, scale=0.125)
                        es.append(e)

                    def v_half(t):
                        for vh in range(2):    # halves of V feature dim
                            vp = psMM.tile([128, 2, N], F32, tag="mm", name="vp")
                            for s in range(2):
                                vq = 2 * vh + s
                                for k in range(KC):
                                    nc.tensor.matmul(
                                        vp[:, s, :],
                                        xnT[:, k, :, t * 128:(t + 1) * 128],
                                        _pair(wqkv_sb[:, k, 2 * C + vq * 256:2 * C + (vq + 1) * 256]),
                                        start=(s == 0 and k == 0),
                                        stop=(s == 1 and k == KC - 1),
                                        perf_mode=PM.DoubleRow,
                                    )
                            nc.vector.tensor_scalar(
                                v_sb[:, t, vh * 8:(vh + 1) * 8, 0:DH],
                                vp.rearrange("p s (a d) -> p (s a) d", d=DH),
                                1.0 / 64, None, ALU.mult,
                            )

                    for h in range(H // 2):
                        s_head(h)
                    v_half(0)
                    for h in range(H // 2, H):
                        s_head(h)
                    v_half(1)
                    return es, v_sb

                def stage3(b, es, v_sb):
                    """AV + normalize, O transpose, proj + residual, LN2 stats."""
                    o_sb = pa1.tile([128, 2, H, DH], BF16, tag="o", name="o_sb")
                    rd = pa1.tile([128, 2, H], F32, tag="rd", name="rd")
                    oT = pa1.tile([128, KC, 2, N], F8, tag="oT", name="oT")
                    for qc in range(2):
                        for hg in range(4):
                            av = psAV.tile([128, 4, DH + 1], F32, tag="av", name="av")
                            for hh in range(4):
                                h = hg * 4 + hh
                                for nk in range(2):
                                    nc.tensor.matmul(
                                        av[:, hh, :],
                                        es[h][:, nk, qc * 128:(qc + 1) * 128],
                                        v_sb[:, nk, h, :],
                                        start=(hh == 0 and nk == 0),
                                        stop=(hh == 3 and nk == 1),
                                    )
                            nc.vector.reciprocal(
                                rd[:, qc, hg * 4:(hg + 1) * 4], av[:, :, DH]
                            )
                            for hh in range(4):
                                h = hg * 4 + hh
                                nc.vector.tensor_scalar(
                                    o_sb[:, qc, h, :], av[:, hh, 0:DH],
                                    rd[:, qc, h:h + 1], None, ALU.mult,
                                )
                        for cg in range(2):
                            tp = tp4("tp", [
                                o_sb[:, qc, 2 * (cg * 4 + j):2 * (cg * 4 + j) + 2, :]
                                .rearrange("p a d -> p (a d)")
                                for j in range(4)
                            ], identb)
                            hi = oT[:, cg * 4:(cg + 1) * 4, 0, qc * 128:(qc + 1) * 128]
                            nc.scalar.copy(hi, tp)
                            nc.vector.scalar_tensor_tensor(
                                oT[:, cg * 4:(cg + 1) * 4, 1, qc * 128:(qc + 1) * 128],
                                tp, 1.0, hi, ALU.mult, ALU.subtract,
                            )

                    for t in range(2):
                        for ch in range(2):    # output halves of 512
                            pp = psMM.tile([128, 2, N], F32, tag="mm", name="pp")
                            for s in range(2):
                                cq = 2 * ch + s
                                for k in range(KC):
                                    nc.tensor.matmul(
                                        pp[:, s, :],
                                        oT[:, k, :, t * 128:(t + 1) * 128],
                                        _pair(wp_sb[:, k, cq * 256:(cq + 1) * 256]),
                                        start=(s == 0 and k == 0),
                                        stop=(s == 1 and k == KC - 1),
                                        perf_mode=PM.DoubleRow,
                                    )
                            tt = 2 * b + t
                            nc.vector.scalar_tensor_tensor(
                                xr[:, tt, ch * 512:(ch + 1) * 512],
                                pp.rearrange("p s n -> p (s n)"), 1.0 / 64,
                                xr[:, tt, ch * 512:(ch + 1) * 512],
                                ALU.mult, ALU.add,
                            )
                        tt = 2 * b + t
                        st2 = pa1.tile([128, 2, 6], F32, tag="st2a", name="st2")
                        rv = xr[:, tt, :].rearrange("p (s f) -> p s f", s=2)
                        for s in range(2):
                            nc.vector.bn_stats(st2[:, s, :], rv[:, s, :])
                        nc.vector.bn_aggr(mvs2[:, tt, :], st2)

                # software pipeline: stage3(b-1) slots between stage1(b)
                # and stage2(b) so PE never waits on Act exps / DVE evacs
                carry = None
                for b in range(BLOC):
                    xnT, qkT = stage1(b)
                    if b == 0:
                        ln1_stats(2, NT)
                    if carry is not None:
                        stage3(b - 1, *carry)
                    carry = stage2(b, xnT, qkT)
                stage3(BLOC - 1, *carry)

            # ---------------- phase B: MLP ----------------------------------
            with (
                tc.tile_pool(name="pbst", bufs=2) as pbst,
                tc.tile_pool(name="pbw", bufs=2) as pbw,
                tc.tile_pool(name="pbh", bufs=1) as pbh,
                tc.tile_pool(name="pb1", bufs=2) as pb1,
                tc.tile_pool(name="psT2", bufs=2, space="PSUM") as psT2,
                tc.tile_pool(name="psF1", bufs=3, space="PSUM") as psF1,
                tc.tile_pool(name="psF2", bufs=2, space="PSUM") as psF2,
            ):
                # LN2 stats were computed per-batch in phase A
                sq2 = pbst.tile([128, NT], F32, tag="sq2")
                nc.scalar.activation(sq2, mvs2[:, :, 1], AF.Sqrt, bias=eps_sb, scale=1.0)
                nc.vector.reciprocal(rstds2, sq2)

                NB = 2
                BT = TOK // NB          # 1024 tokens per block
                for blk in range(NB):
                    xnT2 = pbh.tile([128, KC, 2, BT], F8, tag="xnT2")
                    hT = pbh.tile([128, KF, BT], F8, tag="hT")

                    # prefetch fc1 weights before the LN2 norm/transpose work
                    w1tiles = {}

                    def w1_fetch(sl):
                        w1hc = pbw.tile([128, KC, 512], F8, tag="w1hc", name="w1hc")
                        nc.sync.dma_start(w1hc, w1hi_d[:, :, sl * 512:(sl + 1) * 512])
                        w1lc = pbw.tile([128, KC, 512], F8, tag="w1lc", name="w1lc")
                        nc.sync.dma_start(w1lc, w1lo_d[:, :, sl * 512:(sl + 1) * 512])
                        w1tiles[sl] = (w1hc, w1lc)

                    w1_fetch(0)
                    for tt in range(BT // 128):
                        t = blk * (BT // 128) + tt
                        xn2 = pb1.tile([128, C], BF16, tag="xn2")
                        nc.vector.tensor_scalar(
                            xn2, xr[:, t, :], mvs2[:, t, 0:1], rstds2[:, t:t + 1],
                            ALU.subtract, ALU.mult,
                        )
                        for cg in range(2):
                            tp = psT2.tile([128, 4, 128], BF16, tag="tp2")
                            for j in range(4):
                                nc.tensor.matmul(
                                    tp[:, j, :],
                                    xn2[:, (cg * 4 + j) * 128:(cg * 4 + j + 1) * 128],
                                    identb, is_transpose=True,
                                    start=(j == 0), stop=(j == 3),
                                )
                            hi = xnT2[:, cg * 4:(cg + 1) * 4, 0, tt * 128:(tt + 1) * 128]
                            nc.scalar.copy(hi, tp)
                            nc.vector.scalar_tensor_tensor(
                                xnT2[:, cg * 4:(cg + 1) * 4, 1, tt * 128:(tt + 1) * 128],
                                tp, 1.0, hi, ALU.mult, ALU.subtract,
                            )

                    # fc1: both-dual via 3 single-DR passes
                    # (w_hi@x_hi + w_lo@x_hi + w_hi@x_lo), gelu -> fp8 hT
                    for sl in range(8):     # dff slices of 512
                        if sl + 1 < 8:
                            w1_fetch(sl + 1)
                        w1hc, w1lc = w1tiles.pop(sl)
                        for dc in range(4):
                            for tc2 in range(2):
                                fp1 = psF1.tile([128, 2, 256], F32, tag="f1")
                                passes = [(w1hc, 0), (w1lc, 0), (w1hc, 1)]
                                for th in range(2):
                                    tq = 2 * tc2 + th
                                    for pi, (wt, xi) in enumerate(passes):
                                        for kp in range(KC // 2):
                                            nc.tensor.matmul(
                                                fp1[:, th, :],
                                                wt[:, 2 * kp:2 * kp + 2,
                                                   dc * 128:(dc + 1) * 128],
                                                xnT2[:, 2 * kp:2 * kp + 2, xi,
                                                     tq * 256:(tq + 1) * 256],
                                                start=(th == 0 and pi == 0 and kp == 0),
                                                stop=(th == 1 and pi == 2
                                                      and kp == KC // 2 - 1),
                                                perf_mode=PM.DoubleRow,
                                            )
                                nc.scalar.activation(
                                    hT[:, sl * 4 + dc, tc2 * 512:(tc2 + 1) * 512],
                                    fp1.rearrange("p a b -> p (a b)"),
                                    AF.Gelu_apprx_tanh, scale=1.0 / 64,
                                )

                    # fc2: w-dual DoubleRow + residual -> y
                    for co in range(4):     # output quarters of 256
                        w2c = pbw.tile([128, KF, 2, 256], F8, tag="w2c")
                        nc.sync.dma_start(w2c, w2_d[:, :, :, co * 256:(co + 1) * 256])
                        for tcc in range(BT // 128):
                            t = blk * (BT // 128) + tcc
                            fp2 = psF2.tile([128, 256], F32, tag="f2")
                            for k in range(KF):
                                nc.tensor.matmul(
                                    fp2,
                                    _pair(hT[:, k, tcc * 128:(tcc + 1) * 128]),
                                    w2c[:, k, :, :],
                                    start=(k == 0), stop=(k == KF - 1),
                                    perf_mode=PM.DoubleRow,
                                )
                            nc.vector.scalar_tensor_tensor(
                                xr[:, t, co * 256:(co + 1) * 256], fp2, 1.0 / 64,
                                xr[:, t, co * 256:(co + 1) * 256],
                                ALU.mult, ALU.add,
                            )
                            if co == 3:
                                # final quarter done: ship the tile while the
                                # next tiles still compute
                                nc.sync.dma_start(
                                    y_d[t * 128:(t + 1) * 128, :], xr[:, t, :]
                                )

            pst_ctx.__exit__(None, None, None)

    nc.finalize()
    return nc


_NC_CACHE = {}


def _get_nc():
    if "nc" not in _NC_CACHE:
        _NC_CACHE["nc"] = build()
    return _NC_CACHE["nc"]


def _q8(x):
    return np.clip(np.asarray(x, np.float32), -240, 240).astype(NP8)


def kernel(**inputs):
    x = np.asarray(inputs["x"], dtype=np.float32)
    qkv_w = np.asarray(inputs["qkv_w"], dtype=np.float32)
    proj_w = np.asarray(inputs["proj_w"], dtype=np.float32)
    fc1_w = np.asarray(inputs["fc1_w"], dtype=np.float32)
    fc2_w = np.asarray(inputs["fc2_w"], dtype=np.float32)
    ln1_g = np.asarray(inputs["ln1_g"], dtype=np.float32)
    ln2_g = np.asarray(inputs["ln2_g"], dtype=np.float32)
    rel_pos_bias = np.asarray(inputs["rel_pos_bias"], dtype=np.float32)
    rel_pos_idx = np.asarray(inputs["rel_pos_idx"])

    for name in ("qkv_b", "proj_b", "fc1_b", "fc2_b", "ln1_b", "ln2_b"):
        assert not np.any(np.asarray(inputs[name])), f"nonzero {name} unsupported"

    wqkv = (ln1_g[:, None] * qkv_w).reshape(KC, 128, 3 * C).transpose(1, 0, 2)
    wqkv8 = _q8(64 * wqkv)
    wp = proj_w.reshape(KC, 128, C).transpose(1, 0, 2)
    wp8 = _q8(64 * wp)
    w1 = (ln2_g[:, None] * fc1_w).reshape(KC, 128, DFF).transpose(1, 0, 2)
    w1_hi = _q8(64 * w1)
    w1_lo = _q8(64 * w1 - w1_hi.astype(np.float32))
    w2 = fc2_w.reshape(KF, 128, C).transpose(1, 0, 2)   # [128, KF, C]
    w2_hi = _q8(64 * w2)
    w2_lo = _q8(64 * w2 - w2_hi.astype(np.float32))
    w2d = np.ascontiguousarray(np.stack([w2_hi, w2_lo], axis=2))  # [128,KF,2,C]

    # device multiplies by 8*I (DoubleRow pair) and exp applies scale 1/8,
    # so store a dual-fp8 split of B itself: [128, 2(hi/lo), 2(nk), H, q]
    Bm = rel_pos_bias[rel_pos_idx].reshape(N, N, H)          # [q, k, h]
    btf = np.ascontiguousarray(
        Bm.transpose(1, 2, 0).reshape(2, 128, H, N).transpose(1, 0, 2, 3)
    ).astype(np.float32)                                     # [128, 2(nk), H, q]
    bt_hi = _q8(btf)
    bt_lo = _q8(btf - bt_hi.astype(np.float32))
    bt = np.ascontiguousarray(np.stack([bt_hi, bt_lo], axis=1))

    nc = _get_nc()
    in_maps = []
    for c in range(NCORES):
        xs = np.ascontiguousarray(
            x[c * BLOC:(c + 1) * BLOC].reshape(TOK, C)
        ).astype(np.float32)
        in_maps.append(
            dict(x=xs, wqkv=wqkv8, wp=wp8, bt=bt, w1hi=w1_hi, w1lo=w1_lo, w2=w2d)
        )
    res = run_bass_kernel_spmd(nc, in_maps, core_ids=list(range(NCORES)))
    y = np.concatenate([res.results[c]["y"] for c in range(NCORES)], axis=0)
    return y.reshape(B, N, C).astype(np.float32)


# revision 35
# speedup vs baseline: 1.0316x; 1.0316x over previous
"""LITv1 transformer block on 8 TRN2 NeuronCores, data-parallel over batch.

v2: fp8 DoubleRow matmuls with error-feedback dual-fp8 operands.

Per-core layout (8 batches x 256 tokens):
- x/r1 residual stream resident in SBUF f32 (one 8MB buffer, updated in place)
- LN stats batched per phase (one Act Sqrt per phase -> few act-table loads)
- QKV: x-dual fp8 DoubleRow (xn hi+lo pairs in the moving slots, single-fp8
  weights broadcast stride-0 into the stationary slots) -> 0.25 cyc/k-tile
- attention internals bf16: S^T = K^T.T @ Q^T with the relative-position bias
  accumulated into PSUM via a dual-fp8 DoubleRow 8*I matmul; exp on Act ->
  bf16 P; token-major AV with a ones-column for the softmax denominator
- proj: o-dual fp8 DoubleRow stationary, single-fp8 weights broadcast
- MLP: fc1 both-dual via 3 single-DR passes (w_hi@x_hi + w_lo@x_hi +
  w_hi@x_lo); gelu -> fp8 h; fc2 w-dual DoubleRow (W2 hi+lo streamed)
- all transposes in bf16 via PE identity matmuls; dual-fp8 splits happen at
  the PSUM evacuation (hi = copy, lo = psum - hi via scalar_tensor_tensor)
- software-pipelined phase A (stage3 of batch b-1 between stage1/stage2 of
  batch b) so PE never waits on Act exps or DVE evacuations
"""
import sys

import numpy as np

sys.path.insert(0, "/opt/trn_rl_repo")

import ml_dtypes  # noqa: E402

import concourse.bass as bass  # noqa: E402
import concourse.mybir as mybir  # noqa: E402
import concourse.tile as tile  # noqa: E402
from concourse import bacc  # noqa: E402
from concourse.bass_utils import run_bass_kernel_spmd  # noqa: E402
from concourse.masks import make_identity  # noqa: E402

F32 = mybir.dt.float32
F8 = mybir.dt.float8e4
BF16 = mybir.dt.bfloat16
AF = mybir.ActivationFunctionType
ALU = mybir.AluOpType
PM = mybir.MatmulPerfMode

NP8 = ml_dtypes.float8_e4m3
NPB = ml_dtypes.bfloat16

B, N, C = 64, 256, 1024
H, DH = 16, 64
DFF = 4 * C
NCORES = 8
BLOC = B // NCORES          # 8 batches per core
TOK = BLOC * N              # 2048 tokens per core
NT = TOK // 128             # 16 token tiles of 128
KC = C // 128               # 8 contraction chunks of 128
KF = DFF // 128             # 32 dff chunks


def _pair(ap):
    """Broadcast a [128, X] AP to [128, 2, X] with stride-0 pair dim."""
    return ap.unsqueeze(1).broadcast_to([ap.shape[0], 2, ap.shape[-1]])


def build():
    nc = bacc.Bacc("TRN2")
    x_d = nc.dram_tensor("x", [TOK, C], F32, kind="ExternalInput")
    wqkv_d = nc.dram_tensor("wqkv", [128, KC, 3 * C], F8, kind="ExternalInput")
    wp_d = nc.dram_tensor("wp", [128, KC, C], F8, kind="ExternalInput")
    bt_d = nc.dram_tensor("bt", [128, 2, 2, H, N], F8, kind="ExternalInput")
    w1hi_d = nc.dram_tensor("w1hi", [128, KC, DFF], F8, kind="ExternalInput")
    w1lo_d = nc.dram_tensor("w1lo", [128, KC, DFF], F8, kind="ExternalInput")
    w2_d = nc.dram_tensor("w2", [128, KF, 2, C], F8, kind="ExternalInput")
    y_d = nc.dram_tensor("y", [TOK, C], F32, kind="ExternalOutput")

    with tile.TileContext(nc) as tc:
        with (
            tc.tile_pool(name="consts", bufs=1) as consts,
            tc.tile_pool(name="resid", bufs=1) as resid,
        ):
            ident_f = consts.tile([128, 128], F32)
            make_identity(nc, ident_f)
            identb = consts.tile([128, 128], BF16)
            nc.vector.tensor_copy(identb, ident_f)
            eight8 = consts.tile([128, 128], F8)
            nc.vector.tensor_scalar(eight8, ident_f, 8.0, None, ALU.mult)
            eps_sb = consts.tile([128, 1], F32)
            nc.vector.memset(eps_sb, 1e-5)

            xr = resid.tile([128, NT, C], F32)      # x then r1 then y, in place
            mvs = resid.tile([128, NT, 2], F32)     # LN1 mean/var
            rstds = resid.tile([128, NT], F32)
            mvs2 = resid.tile([128, NT, 2], F32)    # LN2 mean/var
            rstds2 = resid.tile([128, NT], F32)

            # ---------------- prologue: load x, LN1 stats --------------------
            pst_ctx = tc.tile_pool(name="pst", bufs=2)
            pst = pst_ctx.__enter__()
            sq = pst.tile([128, NT], F32, tag="sq")

            def ln1_stats(lo, hi_t):
                for t in range(lo, hi_t):
                    st = pst.tile([128, 2, 6], F32, tag="st", name="st")
                    xv = xr[:, t, :].rearrange("p (s f) -> p s f", s=2)
                    for s in range(2):
                        nc.vector.bn_stats(st[:, s, :], xv[:, s, :])
                    nc.vector.bn_aggr(mvs[:, t, :], st)
                nc.scalar.activation(
                    sq[:, lo:hi_t], mvs[:, lo:hi_t, 1], AF.Sqrt,
                    bias=eps_sb, scale=1.0,
                )
                nc.vector.reciprocal(rstds[:, lo:hi_t], sq[:, lo:hi_t])

            # first 4 x tiles unblock batches 0-1 quickly
            for t in range(4):
                nc.sync.dma_start(xr[:, t, :], x_d[t * 128:(t + 1) * 128, :])
            ln1_stats(0, 4)

            # ---------------- phase A: attention + proj ----------------------
            with (
                tc.tile_pool(name="paw", bufs=1) as paw,
                tc.tile_pool(name="pa1", bufs=2) as pa1,
                tc.tile_pool(name="pa2", bufs=2) as pa2,
                tc.tile_pool(name="pae", bufs=16) as pae,
                tc.tile_pool(name="psT", bufs=2, space="PSUM") as psT,
                tc.tile_pool(name="psQK", bufs=1, space="PSUM") as psQK,
                tc.tile_pool(name="psMM", bufs=1, space="PSUM") as psMM,
                tc.tile_pool(name="psS", bufs=2, space="PSUM") as psS,
                tc.tile_pool(name="psAV", bufs=2, space="PSUM") as psAV,
            ):
                # DMA order: wqkv before remaining x tiles so batch 0's QKV
                # isn't blocked; bt before wp (bias needed earlier than proj)
                wqkv_sb = paw.tile([128, KC, 3 * C], F8)
                nc.sync.dma_start(wqkv_sb, wqkv_d[:])
                bt_sb = paw.tile([128, 2, 2, H, N], F8)
                nc.sync.dma_start(bt_sb, bt_d[:])
                wp_sb = paw.tile([128, KC, C], F8)
                nc.sync.dma_start(wp_sb, wp_d[:])
                for t in range(4, NT):
                    nc.sync.dma_start(xr[:, t, :], x_d[t * 128:(t + 1) * 128, :])

                def tp4(pool_tag, srcs, ident):
                    """4 transposes into one PSUM bank as ONE accumulation
                    group (hardware zeroes PSUM lazily per 2KB region; separate
                    groups in one region would wipe earlier sub-tiles)."""
                    tp = psT.tile([128, 4, 128], BF16, tag=pool_tag)
                    for j, src in enumerate(srcs):
                        nc.tensor.matmul(
                            tp[:, j, :], src, ident, is_transpose=True,
                            start=(j == 0), stop=(j == 3),
                        )
                    return tp

                def stage1(b):
                    """LN1 norm/transpose/dual-evac + QKV(q,k) matmuls."""
                    xn16 = pa1.tile([128, 2, C], BF16, tag="xn16", name="xn16")
                    xnT = pa1.tile([128, KC, 2, N], F8, tag="xnT", name="xnT")
                    for i in range(2):
                        t = 2 * b + i
                        nc.vector.tensor_scalar(
                            xn16[:, i, :], xr[:, t, :],
                            mvs[:, t, 0:1], rstds[:, t:t + 1],
                            ALU.subtract, ALU.mult,
                        )
                        for cg in range(2):
                            tp = tp4("tp", [
                                xn16[:, i, (cg * 4 + j) * 128:(cg * 4 + j + 1) * 128]
                                for j in range(4)
                            ], identb)
                            hi = xnT[:, cg * 4:(cg + 1) * 4, 0, i * 128:(i + 1) * 128]
                            nc.scalar.copy(hi, tp)
                            nc.vector.scalar_tensor_tensor(
                                xnT[:, cg * 4:(cg + 1) * 4, 1, i * 128:(i + 1) * 128],
                                tp, 1.0, hi, ALU.mult, ALU.subtract,
                            )

                    qkT = pa2.tile([128, 2 * KC, N], BF16, tag="qkT", name="qkT")
                    for fp in range(KC):       # pairs of 128-feat chunks (Q,K)
                        qp = psQK.tile([128, 2, N], F32, tag="qk", name="qp")
                        for s in range(2):
                            fo = 2 * fp + s
                            for k in range(KC):
                                nc.tensor.matmul(
                                    qp[:, s, :],
                                    _pair(wqkv_sb[:, k, fo * 128:(fo + 1) * 128]),
                                    xnT[:, k, :, :],
                                    start=(s == 0 and k == 0),
                                    stop=(s == 1 and k == KC - 1),
                                    perf_mode=PM.DoubleRow,
                                )
                        nc.scalar.activation(
                            qkT[:, 2 * fp:2 * fp + 2, :], qp, AF.Copy,
                            scale=1.0 / 64,
                        )
                    return xnT, qkT

                def stage2(b, xnT, qkT):
                    """S^T + dual-fp8 bias + exp -> bf16 P, with the V matmuls
                    interleaved so PE has work while Act drains the exps."""
                    es = []
                    v_sb = pa2.tile([128, 2, H, DH + 1], BF16, tag="v", name="v_sb")
                    nc.vector.memset(v_sb[:, :, :, DH:DH + 1], 1.0)

                    def s_head(h):
                        sp = psS.tile([128, 2, N], F32, tag="sp", name="sp")
                        r0 = (h % 2) * 64
                        for nk in range(2):
                            nc.tensor.matmul(
                                sp[:, nk, :], _pair(eight8),
                                bt_sb[:, :, nk, h, :],
                                start=(nk == 0), stop=False,
                                perf_mode=PM.DoubleRow,
                            )
                            nc.tensor.matmul(
                                sp[:, nk, :],
                                qkT[r0:r0 + 64, KC + h // 2, nk * 128:(nk + 1) * 128],
                                qkT[r0:r0 + 64, h // 2, :],
                                start=False, stop=(nk == 1),
                            )
                        e = pae.tile([128, 2, N], BF16, tag="e", name="e")
                        nc.scalar.activation(e, sp, AF.Exp, bias=0.0, scale=0.125)
                        es.append(e)

                    def v_half(t):
                        for vh in range(2):    # halves of V feature dim
                            vp = psMM.tile([128, 2, N], F32, tag="mm", name="vp")
                            for s in range(2):
                                vq = 2 * vh + s
                                for k in range(KC):
                                    nc.tensor.matmul(
                                        vp[:, s, :],
                                        xnT[:, k, :, t * 128:(t + 1) * 128],
                                        _pair(wqkv_sb[:, k, 2 * C + vq * 256:2 * C + (vq + 1) * 256]),
                                        start=(s == 0 and k == 0),
                                        stop=(s == 1 and k == KC - 1),
                                        perf_mode=PM.DoubleRow,
                                    )
                            nc.vector.tensor_scalar(
                                v_sb[:, t, vh * 8:(vh + 1) * 8, 0:DH],
                                vp.rearrange("p s (a d) -> p (s a) d", d=DH),
                                1.0 / 64, None, ALU.mult,
                            )

                    for h in range(H // 2):
                        s_head(h)
                    v_half(0)
                    for h in range(H // 2, H):
                        s_head(h)
                    v_half(1)
                    return es, v_sb

                def stage3(b, es, v_sb):
                    """AV + normalize, O transpose, proj + residual, LN2 stats."""
                    o_sb = pa1.tile([128, 2, H, DH], BF16, tag="o", name="o_sb")
                    rd = pa1.tile([128, 2, H], F32, tag="rd", name="rd")
                    oT = pa1.tile([128, KC, 2, N], F8, tag="oT", name="oT")
                    for qc in range(2):
                        for hg in range(4):
                            av = psAV.tile([128, 4, DH + 1], F32, tag="av", name="av")
                            for hh in range(4):
                                h = hg * 4 + hh
                                for nk in range(2):
                                    nc.tensor.matmul(
                                        av[:, hh, :],
                                        es[h][:, nk, qc * 128:(qc + 1) * 128],
                                        v_sb[:, nk, h, :],
                                        start=(hh == 0 and nk == 0),
                                        stop=(hh == 3 and nk == 1),
                                    )
                            nc.vector.reciprocal(
                                rd[:, qc, hg * 4:(hg + 1) * 4], av[:, :, DH]
                            )
                            for hh in range(4):
                                h = hg * 4 + hh
                                nc.vector.tensor_scalar(
                                    o_sb[:, qc, h, :], av[:, hh, 0:DH],
                                    rd[:, qc, h:h + 1], None, ALU.mult,
                                )
                        for cg in range(2):
                            tp = tp4("tp", [
                                o_sb[:, qc, 2 * (cg * 4 + j):2 * (cg * 4 + j) + 2, :]
                                .rearrange("p a d -> p (a d)")
                                for j in range(4)
                            ], identb)
                            hi = oT[:, cg * 4:(cg + 1) * 4, 0, qc * 128:(qc + 1) * 128]
                            nc.scalar.copy(hi, tp)
                            nc.vector.scalar_tensor_tensor(
                                oT[:, cg * 4:(cg + 1) * 4, 1, qc * 128:(qc + 1) * 128],
                                tp, 1.0, hi, ALU.mult, ALU.subtract,
                            )

                    for t in range(2):
                        for ch in range(2):    # output halves of 512
                            pp = psMM.tile([128, 2, N], F32, tag="mm", name="pp")
                            for s in range(2):
                                cq = 2 * ch + s
                                for k in range(KC):
                                    nc.tensor.matmul(
                                        pp[:, s, :],
                                        oT[:, k, :, t * 128:(t + 1) * 128],
                                        _pair(wp_sb[:, k, cq * 256:(cq + 1) * 256]),
                                        start=(s == 0 and k == 0),
                                        stop=(s == 1 and k == KC - 1),
                                        perf_mode=PM.DoubleRow,
                                    )
                            tt = 2 * b + t
                            nc.vector.scalar_tensor_tensor(
                                xr[:, tt, ch * 512:(ch + 1) * 512],
                                pp.rearrange("p s n -> p (s n)"), 1.0 / 64,
                                xr[:, tt, ch * 512:(ch + 1) * 512],
                                ALU.mult, ALU.add,
                            )
                        tt = 2 * b + t
                        st2 = pa1.tile([128, 2, 6], F32, tag="st2a", name="st2")
                        rv = xr[:, tt, :].rearrange("p (s f) -> p s f", s=2)
                        for s in range(2):
                            nc.vector.bn_stats(st2[:, s, :], rv[:, s, :])
                        nc.vector.bn_aggr(mvs2[:, tt, :], st2)

                # software pipeline: stage3(b-1) slots between stage1(b)
                # and stage2(b) so PE never waits on Act exps / DVE evacs
                carry = None
                for b in range(BLOC):
                    xnT, qkT = stage1(b)
                    if b == 0:
                        ln1_stats(4, NT)
                    if carry is not None:
                        stage3(b - 1, *carry)
                    carry = stage2(b, xnT, qkT)
                stage3(BLOC - 1, *carry)

            # ---------------- phase B: MLP ----------------------------------
            with (
                tc.tile_pool(name="pbst", bufs=2) as pbst,
                tc.tile_pool(name="pbw", bufs=2) as pbw,
                tc.tile_pool(name="pbh", bufs=1) as pbh,
                tc.tile_pool(name="pb1", bufs=2) as pb1,
                tc.tile_pool(name="psT2", bufs=2, space="PSUM") as psT2,
                tc.tile_pool(name="psF1", bufs=3, space="PSUM") as psF1,
                tc.tile_pool(name="psF2", bufs=2, space="PSUM") as psF2,
            ):
                # LN2 stats were computed per-batch in phase A
                sq2 = pbst.tile([128, NT], F32, tag="sq2")
                nc.scalar.activation(sq2, mvs2[:, :, 1], AF.Sqrt, bias=eps_sb, scale=1.0)
                nc.vector.reciprocal(rstds2, sq2)

                NB = 2
                BT = TOK // NB          # 1024 tokens per block
                for blk in range(NB):
                    xnT2 = pbh.tile([128, KC, 2, BT], F8, tag="xnT2")
                    hT = pbh.tile([128, KF, BT], F8, tag="hT")

                    # prefetch fc1 weights before the LN2 norm/transpose work
                    w1tiles = {}

                    def w1_fetch(sl):
                        w1hc = pbw.tile([128, KC, 512], F8, tag="w1hc", name="w1hc")
                        nc.sync.dma_start(w1hc, w1hi_d[:, :, sl * 512:(sl + 1) * 512])
                        w1lc = pbw.tile([128, KC, 512], F8, tag="w1lc", name="w1lc")
                        nc.sync.dma_start(w1lc, w1lo_d[:, :, sl * 512:(sl + 1) * 512])
                        w1tiles[sl] = (w1hc, w1lc)

                    w1_fetch(0)
                    for tt in range(BT // 128):
                        t = blk * (BT // 128) + tt
                        xn2 = pb1.tile([128, C], BF16, tag="xn2")
                        nc.vector.tensor_scalar(
                            xn2, xr[:, t, :], mvs2[:, t, 0:1], rstds2[:, t:t + 1],
                            ALU.subtract, ALU.mult,
                        )
                        for cg in range(2):
                            tp = psT2.tile([128, 4, 128], BF16, tag="tp2")
                            for j in range(4):
                                nc.tensor.matmul(
                                    tp[:, j, :],
                                    xn2[:, (cg * 4 + j) * 128:(cg * 4 + j + 1) * 128],
                                    identb, is_transpose=True,
                                    start=(j == 0), stop=(j == 3),
                                )
                            hi = xnT2[:, cg * 4:(cg + 1) * 4, 0, tt * 128:(tt + 1) * 128]
                            nc.scalar.copy(hi, tp)
                            nc.vector.scalar_tensor_tensor(
                                xnT2[:, cg * 4:(cg + 1) * 4, 1, tt * 128:(tt + 1) * 128],
                                tp, 1.0, hi, ALU.mult, ALU.subtract,
                            )

                    # fc1: both-dual via 3 single-DR passes
                    # (w_hi@x_hi + w_lo@x_hi + w_hi@x_lo), gelu -> fp8 hT
                    for sl in range(8):     # dff slices of 512
                        if sl + 1 < 8:
                            w1_fetch(sl + 1)
                        w1hc, w1lc = w1tiles.pop(sl)
                        for dc in range(4):
                            for tc2 in range(2):
                                fp1 = psF1.tile([128, 2, 256], F32, tag="f1")
                                passes = [(w1hc, 0), (w1lc, 0), (w1hc, 1)]
                                for th in range(2):
                                    tq = 2 * tc2 + th
                                    for pi, (wt, xi) in enumerate(passes):
                                        for kp in range(KC // 2):
                                            nc.tensor.matmul(
                                                fp1[:, th, :],
                                                wt[:, 2 * kp:2 * kp + 2,
                                                   dc * 128:(dc + 1) * 128],
                                                xnT2[:, 2 * kp:2 * kp + 2, xi,
                                                     tq * 256:(tq + 1) * 256],
                                                start=(th == 0 and pi == 0 and kp == 0),
                                                stop=(th == 1 and pi == 2
                                                      and kp == KC // 2 - 1),
                                                perf_mode=PM.DoubleRow,
                                            )
                                nc.scalar.activation(
                                    hT[:, sl * 4 + dc, tc2 * 512:(tc2 + 1) * 512],
                                    fp1.rearrange("p a b -> p (a b)"),
                                    AF.Gelu_apprx_tanh, scale=1.0 / 64,
                                )

                    # fc2: w-dual DoubleRow + residual -> y
                    for co in range(4):     # output quarters of 256
                        w2c = pbw.tile([128, KF, 2, 256], F8, tag="w2c")
                        nc.sync.dma_start(w2c, w2_d[:, :, :, co * 256:(co + 1) * 256])
                        for tcc in range(BT // 128):
                            t = blk * (BT // 128) + tcc
                            fp2 = psF2.tile([128, 256], F32, tag="f2")
                            for k in range(KF):
                                nc.tensor.matmul(
                                    fp2,
                                    _pair(hT[:, k, tcc * 128:(tcc + 1) * 128]),
                                    w2c[:, k, :, :],
                                    start=(k == 0), stop=(k == KF - 1),
                                    perf_mode=PM.DoubleRow,
                                )
                            nc.vector.scalar_tensor_tensor(
                                xr[:, t, co * 256:(co + 1) * 256], fp2, 1.0 / 64,
                                xr[:, t, co * 256:(co + 1) * 256],
                                ALU.mult, ALU.add,
                            )
                            if co == 3:
                                # final quarter done: ship the tile while the
                                # next tiles still compute
                                nc.sync.dma_start(
                                    y_d[t * 128:(t + 1) * 128, :], xr[:, t, :]
                                )

            pst_ctx.__exit__(None, None, None)

    nc.finalize()
    return nc


_NC_CACHE = {}


def _get_nc():
    if "nc" not in _NC_CACHE:
        _NC_CACHE["nc"] = build()
    return _NC_CACHE["nc"]


def _q8(x):
    return np.clip(np.asarray(x, np.float32), -240, 240).astype(NP8)


def kernel(**inputs):
    x = np.asarray(inputs["x"], dtype=np.float32)
    qkv_w = np.asarray(inputs["qkv_w"], dtype=np.float32)
    proj_w = np.asarray(inputs["proj_w"], dtype=np.float32)
    fc1_w = np.asarray(inputs["fc1_w"], dtype=np.float32)
    fc2_w = np.asarray(inputs["fc2_w"], dtype=np.float32)
    ln1_g = np.asarray(inputs["ln1_g"], dtype=np.float32)
    ln2_g = np.asarray(inputs["ln2_g"], dtype=np.float32)
    rel_pos_bias = np.asarray(inputs["rel_pos_bias"], dtype=np.float32)
    rel_pos_idx = np.asarray(inputs["rel_pos_idx"])

    for name in ("qkv_b", "proj_b", "fc1_b", "fc2_b", "ln1_b", "ln2_b"):
        assert not np.any(np.asarray(inputs[name])), f"nonzero {name} unsupported"

    wqkv = (ln1_g[:, None] * qkv_w).reshape(KC, 128, 3 * C).transpose(1, 0, 2)
    wqkv8 = _q8(64 * wqkv)
    wp = proj_w.reshape(KC, 128, C).transpose(1, 0, 2)
    wp8 = _q8(64 * wp)
    w1 = (ln2_g[:, None] * fc1_w).reshape(KC, 128, DFF).transpose(1, 0, 2)
    w1_hi = _q8(64 * w1)
    w1_lo = _q8(64 * w1 - w1_hi.astype(np.float32))
    w2 = fc2_w.reshape(KF, 128, C).transpose(1, 0, 2)   # [128, KF, C]
    w2_hi = _q8(64 * w2)
    w2_lo = _q8(64 * w2 - w2_hi.astype(np.float32))
    w2d = np.ascontiguousarray(np.stack([w2_hi, w2_lo], axis=2))  # [128,KF,2,C]

    # device multiplies by 8*I (DoubleRow pair) and exp applies scale 1/8,
    # so store a dual-fp8 split of B itself: [128, 2(hi/lo), 2(nk), H, q]
    Bm = rel_pos_bias[rel_pos_idx].reshape(N, N, H)          # [q, k, h]
    btf = np.ascontiguousarray(
        Bm.transpose(1, 2, 0).reshape(2, 128, H, N).transpose(1, 0, 2, 3)
    ).astype(np.float32)                                     # [128, 2(nk), H, q]
    bt_hi = _q8(btf)
    bt_lo = _q8(btf - bt_hi.astype(np.float32))
    bt = np.ascontiguousarray(np.stack([bt_hi, bt_lo], axis=1))

    nc = _get_nc()
    in_maps = []
    for c in range(NCORES):
        xs = np.ascontiguousarray(
            x[c * BLOC:(c + 1) * BLOC].reshape(TOK, C)
        ).astype(np.float32)
        in_maps.append(
            dict(x=xs, wqkv=wqkv8, wp=wp8, bt=bt, w1hi=w1_hi, w1lo=w1_lo, w2=w2d)
        )
    res = run_bass_kernel_spmd(nc, in_maps, core_ids=list(range(NCORES)))
    y = np.concatenate([res.results[c]["y"] for c in range(NCORES)], axis=0)
    return y.reshape(B, N, C).astype(np.float32)
